# revision 58
# baseline (speedup 1.0000x reference)
"""Multi-head attention (B=2, N=2048, D=1024, H=16) on 8 trn2 NeuronCores.

Sharding: DP2 (batch) x TP4 (head quarters).  Core c handles batch c//4 and
heads [4*(c%4), 4*(c%4)+4).  Per core:
  - q/k projections as fp8e4 DoubleRow matmuls (weights pre-scaled by 32
    host-side to dodge e4m3 subnormals; x converted bf16->fp8 on the DVE in
    just-in-time blocks).  The v/value path stays bf16 end to end: any fp8
    stage in x->v->z->out passes its ~2% relative error straight through to
    the output, which busts the 2e-2 gate.
  - causal flash-style attention (no max subtraction: scores/32 are tiny so
    exp is safe; softmax denominator comes from a ones-column fused into the
    z-accumulation matmul)
  - a dummy AllGather is issued first so the runtime's global startup
    barrier (tens of us of core-start skew) is absorbed while local DMA and
    compute proceed; then per-(pair, query-block) AllGathers of normalized
    z^T pipeline with attention, each landing as a contiguous [qb] slice of
    one za_all[pr] = [qb, rank, 128, 512] DRAM tensor so a single
    dynamic-offset DMA pulls this core's own query block (no staging pass)
  - output projection for the core's 512-row slice of the sequence
Perf structure: dummy warm-up matmuls ramp the PE p-state through the
input-DMA wait; the PE instruction stream is hand-interleaved (engines
execute in ~emission order): later projection work and the pair-0 output
projection are injected between attention k-tile units, with the pair-0
fillers placed late (0.70+) to ride out peer-skew on the pair-0 gathers.
Host: slices/casts inputs, concatenates the 8 disjoint output slices.
"""

import os
import sys

for _p in ("/opt/trn_rl_repo", "/root/.axon_site/_ro/trn_rl_repo"):
    if os.path.isdir(_p) and _p not in sys.path:
        sys.path.append(_p)

import numpy as np
import ml_dtypes

import concourse.bass as bass
import concourse.mybir as mybir
import concourse.tile as tile
from concourse import bacc

B, N, D, H, HD = 2, 2048, 1024, 16, 64
NCORES, TP = 8, 4
DLOC = D // TP            # 256 local dims (4 heads) per core
P = 128
KT_X = D // P             # 8 contraction tiles for projections
NT = N // P               # 16 n-tiles
QB = 512                  # query block (PSUM bank width in fp32)
NQB = N // QB             # 4
NSLICE = N // TP          # 512 output rows per core
SCALE = 1.0 / 32.0        # 1/sqrt(D)
WS = 32.0                 # fp8 weight pre-scale (avoids e4m3 subnormals)

F32 = mybir.dt.float32
BF16 = mybir.dt.bfloat16
F8 = mybir.dt.float8e4
BF = ml_dtypes.bfloat16
F8NP = ml_dtypes.float8_e4m3
Alu = mybir.AluOpType
Act = mybir.ActivationFunctionType
DR = mybir.MatmulPerfMode.DoubleRow


def build_bass():
    nc = bacc.Bacc("TRN2", num_devices=NCORES)

    xT = nc.dram_tensor("xT", [D, N], BF16, kind="ExternalInput")
    wq = nc.dram_tensor("wq", [D, DLOC], F8, kind="ExternalInput")
    wk = nc.dram_tensor("wk", [D, DLOC], F8, kind="ExternalInput")
    wv = nc.dram_tensor("wv", [D, DLOC], BF16, kind="ExternalInput")
    wo = nc.dram_tensor("wo", [D, D], BF16, kind="ExternalInput")
    bq = nc.dram_tensor("bq", [DLOC], F32, kind="ExternalInput")
    bk = nc.dram_tensor("bk", [DLOC], F32, kind="ExternalInput")
    bv = nc.dram_tensor("bv", [DLOC], F32, kind="ExternalInput")
    bo = nc.dram_tensor("bo", [D], F32, kind="ExternalInput")
    qoff = nc.dram_tensor("qoff", [1, 1], mybir.dt.uint32, kind="ExternalInput")
    # bf16 output: halves the donated zero-buffer upload (start skew) and
    # the tail write DMAs; costs ~0.4% quantization, well inside the gate
    out = nc.dram_tensor("out", [NSLICE, D], BF16, kind="ExternalOutput")

    from concourse.tile_rust import add_dep_helper as _adh
    from concourse.bass import ds

    with tile.TileContext(nc) as tc:
        with (
            tc.tile_pool(name="persist", bufs=1) as persist,
            tc.tile_pool(name="wtp", bufs=4) as wtp,
            tc.tile_pool(name="small", bufs=4) as small,
            tc.tile_pool(name="psA", bufs=2, space="PSUM") as psA,
            tc.tile_pool(name="psS", bufs=2, space="PSUM") as psS,
            tc.tile_pool(name="psZ", bufs=2, space="PSUM") as psZ,
            tc.tile_pool(name="dram", bufs=1, space="DRAM") as dram,
        ):
            # ---- dummy collective first: absorbs the global startup
            # barrier + cc-ring init (up to ~100us of core-start skew)
            # while this core's local DMA + compute proceed, so the real
            # gathers later don't inherit that latency ----
            dum_in = dram.tile([1, 8], BF16, name="dum_in")
            dum_out = dram.tile([TP, 8], BF16, name="dum_out")
            nc.gpsimd.collective_compute(
                "AllGather",
                Alu.bypass,
                replica_groups=[[0, 1, 2, 3], [4, 5, 6, 7]],
                ins=[dum_in[:].opt()],
                outs=[dum_out[:].opt()],
            )

            # ---- PE warm-up: dummy matmuls with no input dependencies
            # ramp the PE p-state while the input DMAs stream; keep it
            # short so the HAM activity budget is spent on real work ----
            warm_l = persist.tile([P, P], BF16)
            nc.vector.memset(warm_l, 0.0)
            warm_r = persist.tile([P, QB], BF16)
            nc.vector.memset(warm_r, 0.0)
            for _ in range(24):
                psw = psA.tile([P, QB], F32, tag="proj", name="psw")
                nc.tensor.matmul(psw, lhsT=warm_l, rhs=warm_r,
                                 start=True, stop=True)

            # preload the EXP activation table before attention needs it
            exp_warm = small.tile([1, 8], F32)
            nc.vector.memset(exp_warm, 0.0)
            exp_wout = small.tile([1, 8], F32)
            nc.scalar.activation(exp_wout, exp_warm, Act.Exp)

            # ---- input loads: wq/wk + the first x block lead the sync
            # queue (they gate the first projection); biases follow ----
            wq_sb = persist.tile([P, KT_X, DLOC], F8)
            wk_sb = persist.tile([P, KT_X, DLOC], F8)
            wq_r = wq[:].rearrange("(kt p) m -> p kt m", p=P)
            wk_r = wk[:].rearrange("(kt p) m -> p kt m", p=P)
            for h in range(2):
                kt4 = slice(h * 4, h * 4 + 4)
                nc.sync.dma_start(wq_sb[:, kt4], wq_r[:, kt4])
                nc.sync.dma_start(wk_sb[:, kt4], wk_r[:, kt4])

            xT_sb = persist.tile([P, KT_X, N], BF16)
            xT_f8 = persist.tile([P, KT_X, N], F8)
            xT_r = xT[:].rearrange("(kt p) m -> p kt m", p=P)

            def load_x_block(qc, chunks=1):
                w = QB // chunks
                for ch in range(chunks):
                    lo = qc * QB + ch * w
                    nc.sync.dma_start(
                        xT_sb[:, :, lo:lo + w], xT_r[:, :, lo:lo + w])

            def conv_x_block(qc):
                # fp8 shadow of x for the DoubleRow q/k projections (the
                # v/value path reads the bf16 original for precision).
                # On DVE, emitted just-in-time before the first consumer
                # so the queue never stalls on a pending x DMA.
                nc.vector.tensor_copy(
                    xT_f8[:, :, qc * QB:(qc + 1) * QB],
                    xT_sb[:, :, qc * QB:(qc + 1) * QB])

            load_x_block(0, chunks=4)
            conv_x_block(0)

            bq_sb = small.tile([P, 2], F32)
            nc.sync.dma_start(bq_sb, bq[:].rearrange("(t p) -> p t", p=P))
            bqs_sb = persist.tile([P, 2], F32)
            nc.vector.tensor_scalar_mul(bqs_sb, bq_sb, SCALE)
            bk_sb = persist.tile([P, 2], F32)
            nc.sync.dma_start(bk_sb, bk[:].rearrange("(t p) -> p t", p=P))
            bv_row = small.tile([1, DLOC], F32)
            nc.sync.dma_start(bv_row, bv[:].rearrange("(a d) -> a d", a=1))
            bv_bc = persist.tile([P, DLOC], F32)
            nc.gpsimd.partition_broadcast(bv_bc, bv_row)
            bo_row = small.tile([1, D], F32)
            nc.sync.dma_start(bo_row, bo[:].rearrange("(a d) -> a d", a=1))
            bo_bc = persist.tile([P, D], F32)
            nc.gpsimd.partition_broadcast(bo_bc, bo_row)

            wv_sb = persist.tile([P, KT_X, DLOC], BF16)
            nc.sync.dma_start(wv_sb, wv[:].rearrange("(kt p) m -> p kt m", p=P))
            for qc in range(1, NQB):
                load_x_block(qc)
            wo_sb = persist.tile([P, KT_X, D], BF16)
            nc.sync.dma_start(wo_sb, wo[:].rearrange("(kt p) m -> p kt m", p=P))

            # 128x128 lower-triangle mask: tri[kk, j] = 1 iff j >= kk
            mask_sb = persist.tile([P, P], BF16)
            nc.gpsimd.memset(mask_sb, 1.0)
            nc.gpsimd.affine_select(
                out=mask_sb,
                in_=mask_sb,
                compare_op=Alu.is_ge,
                fill=0.0,
                base=0,
                pattern=[[1, P]],
                channel_multiplier=-1,
            )

            qT_sb = persist.tile([P, 2, N], BF16)
            kT_sb = persist.tile([P, 2, N], BF16)
            v_sb = persist.tile([P, NT, 4 * 65], BF16)
            # ones columns for the denominators are preset inside fill0
            # (keeps the DVE queue free for the x fp8 convert at start)

            # ---- QKV projection emit-chunks (filler units) ----
            def qk_unit(pr, qc, which):
                if which == "q":
                    psq = psA.tile([P, QB], F32, tag="proj", name="psq")
                    for k2 in range(KT_X // 2):
                        nc.tensor.matmul(
                            psq,
                            lhsT=wq_sb[:, 2 * k2:2 * k2 + 2,
                                       pr * P:(pr + 1) * P],
                            rhs=xT_f8[:, 2 * k2:2 * k2 + 2,
                                      qc * QB:(qc + 1) * QB],
                            start=(k2 == 0),
                            stop=(k2 == KT_X // 2 - 1),
                            perf_mode=DR,
                        )
                    nc.vector.tensor_scalar(
                        qT_sb[:, pr, qc * QB:(qc + 1) * QB],
                        psq, SCALE / WS, bqs_sb[:, pr:pr + 1],
                        Alu.mult, Alu.add,
                    )
                else:
                    psk = psA.tile([P, QB], F32, tag="proj", name="psk")
                    for k2 in range(KT_X // 2):
                        nc.tensor.matmul(
                            psk,
                            lhsT=wk_sb[:, 2 * k2:2 * k2 + 2,
                                       pr * P:(pr + 1) * P],
                            rhs=xT_f8[:, 2 * k2:2 * k2 + 2,
                                      qc * QB:(qc + 1) * QB],
                            start=(k2 == 0),
                            stop=(k2 == KT_X // 2 - 1),
                            perf_mode=DR,
                        )
                    nc.vector.tensor_scalar(
                        kT_sb[:, pr, qc * QB:(qc + 1) * QB],
                        psk, 1.0 / WS, bk_sb[:, pr:pr + 1],
                        Alu.mult, Alu.add,
                    )

            def v_unit(nt):
                psv_full = psA.tile([P, QB], F32, tag="proj", name="psv")
                psv = psv_full[:, :DLOC]
                for kt in range(KT_X):
                    nc.tensor.matmul(
                        psv,
                        lhsT=xT_sb[:, kt, nt * P:(nt + 1) * P],
                        rhs=wv_sb[:, kt, :],
                        start=(kt == 0),
                        stop=(kt == KT_X - 1),
                    )
                nc.vector.tensor_tensor(
                    v_sb[:, nt].rearrange("p (h x) -> p h x", x=65)[:, :, 0:64],
                    psv.rearrange("p (h x) -> p h x", x=64),
                    bv_bc.rearrange("p (h x) -> p h x", x=64),
                    Alu.add,
                )

            # per-core query-block index for the output projection
            qoff_sb = small.tile([1, 1], mybir.dt.uint32)
            nc.sync.dma_start(qoff_sb, qoff[:])
            qregs = nc.alloc_registers()
            nc.regs_load(qregs, qoff_sb[0:1, 0:1])
            qoff_sv = nc.snap(qregs, donate=True)

            # collective buffers.  zin[pr]: [qb, 128, 512] per-block slabs.
            # Per-(pr,qb) AllGathers pipeline with attention; each output
            # lands as a contiguous [qb] slice of za_all[pr]:
            # [qb, rank, 128, 512], so a single dynamic-offset DMA can
            # later pull this core's own query block (no staging pass).
            zin = [dram.tile([NQB, P, QB], BF16, name=f"zin{pr}")
                   for pr in range(2)]
            za_all = [dram.tile([NQB, TP, P, QB], BF16, name=f"za_all{pr}")
                      for pr in range(2)]

            RG = [[0, 1, 2, 3], [4, 5, 6, 7]]

            def gather(pr, qb):
                return nc.gpsimd.collective_compute(
                    "AllGather",
                    Alu.bypass,
                    replica_groups=RG,
                    ins=[zin[pr][qb].opt()],
                    outs=[za_all[pr][qb].opt()],
                )

            # ---- attention for one head pair, with filler injection.
            # `sched` is a sorted list of (when, fn): fn is emitted once the
            # fraction of emitted k-tile slots reaches `when`.  `on_norm` is
            # called (to emit collective triggers / staging) right after a
            # query block's zin writes. ----
            def attention(pr, sched, on_norm):
                nf = len(sched)
                total_slots = sum((qb + 1) * 4 for qb in range(NQB))
                fi = 0
                slot_i = 0
                for qb in range(NQB):
                    kt_max = (qb + 1) * 4
                    zps = [psZ.tile([65, QB], F32, tag="z", name=f"zp{hi}")
                           for hi in range(2)]
                    pending_z = []
                    sp_tiles = wt_tiles = g_os = None
                    for kt in range(kt_max):
                        g, slot = divmod(kt, 2)
                        diag = kt >= qb * 4
                        o = kt * P - qb * QB if diag else 0
                        if slot == 0:
                            sp_tiles = [
                                psS.tile([P, 2, QB], F32, tag="score",
                                         name=f"sp{hi}")
                                for hi in range(2)
                            ]
                            wt_tiles = [
                                wtp.tile([P, 2, QB], BF16, tag="wt",
                                         name=f"wt{hi}")
                                for hi in range(2)
                            ]
                            g_os = []
                        g_os.append(o)
                        s_insts = []
                        for hi in range(2):
                            si = nc.tensor.matmul(
                                sp_tiles[hi][:, slot, o:QB],
                                lhsT=kT_sb[hi * 64:(hi + 1) * 64, pr,
                                           kt * P:(kt + 1) * P],
                                rhs=qT_sb[hi * 64:(hi + 1) * 64, pr,
                                          qb * QB + o:(qb + 1) * QB],
                                start=True,
                                stop=True,
                                tile_position=(hi * 64, 0),
                            )
                            s_insts.append(si)
                        for args in pending_z:
                            zi = nc.tensor.matmul(**args)
                            _adh(zi.ins, s_insts[-1].ins, sync=False,
                                 reason="z after score pair")
                        pending_z = []
                        # inject scheduled filler units between k-tile slots
                        slot_i += 1
                        frac = slot_i / total_slots
                        while fi < nf and sched[fi][0] <= frac:
                            sched[fi][1]()
                            fi += 1

                        if slot == 1:
                            for hi in range(2):
                                if g_os[0] == 0 and g_os[1] == 0:
                                    nc.scalar.activation(
                                        wt_tiles[hi][:, :, :],
                                        sp_tiles[hi][:, :, :], Act.Exp)
                                else:
                                    for s, oo in enumerate(g_os):
                                        nc.scalar.activation(
                                            wt_tiles[hi][:, s, oo:QB],
                                            sp_tiles[hi][:, s, oo:QB],
                                            Act.Exp)
                                for s, oo in enumerate(g_os):
                                    if g * 2 + s >= qb * 4:
                                        nc.vector.tensor_tensor(
                                            wt_tiles[hi][:, s, oo:oo + P],
                                            wt_tiles[hi][:, s, oo:oo + P],
                                            mask_sb,
                                            Alu.mult,
                                        )
                            for s, oo in enumerate(g_os):
                                k_abs = g * 2 + s
                                for hi in range(2):
                                    pending_z.append(dict(
                                        out=zps[hi][:, oo:QB],
                                        lhsT=v_sb[:, k_abs,
                                                  (2 * pr + hi) * 65:
                                                  (2 * pr + hi + 1) * 65],
                                        rhs=wt_tiles[hi][:, s, oo:QB],
                                        start=(k_abs == 0),
                                        stop=(k_abs == kt_max - 1),
                                        skip_group_check=True,
                                    ))
                    for args in pending_z:
                        nc.tensor.matmul(**args)
                    for hi in range(2):
                        # denominator -> SBUF (approx_fast misbehaves on a
                        # PSUM source), reciprocal, gpsimd partition
                        # broadcast, normalize, zin write on the sync queue.
                        den = small.tile([1, QB], F32, tag="den", name="den")
                        nc.vector.tensor_copy(den, zps[hi][64:65, :])
                        recip = small.tile([1, QB], F32, tag="recip",
                                           name="recip")
                        nc.vector.reciprocal_approx_fast(recip, den)
                        rb = small.tile([64, QB], F32, tag="rb", name="rb")
                        nc.gpsimd.partition_broadcast(rb, recip)
                        zn = small.tile([64, QB], BF16, tag="zn", name="zn")
                        nc.vector.tensor_tensor(zn, zps[hi][0:64, :], rb,
                                                Alu.mult)
                        nc.sync.dma_start(
                            zin[pr][qb, hi * 64:(hi + 1) * 64, :], zn)
                    on_norm(pr, qb)
                while fi < nf:
                    sched[fi][1]()
                    fi += 1

            # ---- output projection halves ----
            zg_sb = [persist.tile([P, TP, QB], BF16, name=f"zg{pr}")
                     for pr in range(2)]
            stage_sb = persist.tile([P, NSLICE // P, D // QB, QB], F32)

            def select_zg(pr):
                # dynamic query-block select straight out of the gather
                # outputs (sync queue; gated on the collectives by the
                # tile dependency tracker)
                nc.sync.dma_start(
                    zg_sb[pr],
                    za_all[pr].rearrange("qb r p n -> p qb r n")[
                        :, ds(qoff_sv, 1), :, :].opt(keep_dims=[0, 2, 3]),
                )

            def oproj_unit(pr, mt, oc):
                pso = psA.tile([P, QB], F32, tag="proj", name="pso")
                for r in range(TP):
                    nc.tensor.matmul(
                        pso,
                        lhsT=zg_sb[pr][:, r, mt * P:(mt + 1) * P],
                        rhs=wo_sb[:, pr * TP + r, oc * QB:(oc + 1) * QB],
                        start=(r == 0),
                        stop=(r == TP - 1),
                    )
                if pr == 0:
                    nc.vector.tensor_tensor(
                        stage_sb[:, mt, oc], pso,
                        bo_bc[:, oc * QB:(oc + 1) * QB], Alu.add)
                else:
                    osb = small.tile([P, QB], BF16, tag="osb", name="osb")
                    nc.vector.tensor_tensor(
                        osb, pso, stage_sb[:, mt, oc], Alu.add)
                    nc.sync.dma_start(
                        out[mt * P:(mt + 1) * P, oc * QB:(oc + 1) * QB], osb)

            # ---- emission schedule ----
            qk_unit(0, 0, "q")
            qk_unit(0, 0, "k")

            # fill0 placement: v tile j must land before z consumes it
            # (qb = j//4); q/k for block j before attention reaches qb=j;
            # pair-1 q/k late.
            fill0 = (
                [(0.0005, lambda: nc.vector.memset(v_sb, 1.0))]
                + [(0.001 + 0.018 * nt, lambda nt=nt: v_unit(nt))
                   for nt in range(4)]
                + [
                    (0.06, lambda: conv_x_block(1)),
                    (0.08, lambda: qk_unit(0, 1, "q")),
                    (0.10, lambda: qk_unit(0, 1, "k")),
                    (0.12, lambda: v_unit(4)),
                    (0.14, lambda: v_unit(5)),
                    (0.16, lambda: v_unit(6)),
                    (0.18, lambda: v_unit(7)),
                    (0.19, lambda: conv_x_block(2)),
                    (0.21, lambda: qk_unit(0, 2, "q")),
                    (0.24, lambda: qk_unit(0, 2, "k")),
                    (0.28, lambda: v_unit(8)),
                    (0.31, lambda: v_unit(9)),
                    (0.34, lambda: v_unit(10)),
                    (0.37, lambda: v_unit(11)),
                    (0.40, lambda: conv_x_block(3)),
                    (0.42, lambda: qk_unit(0, 3, "q")),
                    (0.46, lambda: qk_unit(0, 3, "k")),
                    (0.50, lambda: v_unit(12)),
                    (0.53, lambda: v_unit(13)),
                    (0.56, lambda: v_unit(14)),
                    (0.59, lambda: v_unit(15)),
                    (0.66, lambda: qk_unit(1, 0, "q")),
                    (0.72, lambda: qk_unit(1, 0, "k")),
                    (0.80, lambda: qk_unit(1, 1, "q")),
                    (0.88, lambda: qk_unit(1, 1, "k")),
                ]
            )

            def on_norm0(pr, qb):
                gather(0, qb)

            attention(0, fill0, on_norm0)

            fill1 = [
                (0.00, lambda: qk_unit(1, 2, "q")),
                (0.05, lambda: qk_unit(1, 2, "k")),
                (0.22, lambda: qk_unit(1, 3, "q")),
                (0.28, lambda: qk_unit(1, 3, "k")),
                # pr0's last gather rides on peer skew; give it ~2/3 of
                # pr1's attention before the fillers need its output
                (0.66, lambda: select_zg(0)),
            ] + [
                # late placement: by 0.80 of pr1's attention the pr0
                # gathers have cleared even a slow peer, so these never
                # block the in-order PE queue mid-attention
                (0.80 + 0.025 * i,
                 lambda mt=mt, oc=oc: oproj_unit(0, mt, oc))
                for i, (mt, oc) in enumerate(
                    (mt, oc) for mt in range(NSLICE // P)
                    for oc in range(D // QB))
            ]

            def on_norm1(pr, qb):
                gather(1, qb)

            attention(1, fill1, on_norm1)
            select_zg(1)
            # keep the PE p-state hot through the tail gather wait so the
            # final output projection runs at full clock, not half
            for _ in range(45):
                psw = psA.tile([P, QB], F32, tag="proj", name="psw")
                nc.tensor.matmul(psw, lhsT=warm_l, rhs=warm_r,
                                 start=True, stop=True)
            for mt in range(NSLICE // P):
                for oc in range(D // QB):
                    oproj_unit(1, mt, oc)
    nc.compile()
    return nc


def make_in_maps(inputs):
    x = np.asarray(inputs["inputs"], dtype=np.float32)
    ws = {k: np.asarray(inputs[k], dtype=np.float32) for k in
          ("Wq", "Wk", "Wv", "Wo", "bq", "bk", "bv", "bo")}
    # permute Wo rows to the kernel's k-tile order: kt = pr*4 + rank maps to
    # original rows [rank*256 + pr*128, +128)
    wo_perm = (ws["Wo"].reshape(TP, 2, P, D).transpose(1, 0, 2, 3)
               .reshape(D, D))
    wo_bf = np.ascontiguousarray(wo_perm).astype(BF)
    xT_bf = [np.ascontiguousarray(x[b].T).astype(BF) for b in range(B)]
    in_maps = []
    for c in range(NCORES):
        b, q = c // TP, c % TP
        cols = slice(q * DLOC, (q + 1) * DLOC)
        in_maps.append({
            "xT": xT_bf[b],
            "wq": np.ascontiguousarray(ws["Wq"][:, cols] * WS).astype(F8NP),
            "wk": np.ascontiguousarray(ws["Wk"][:, cols] * WS).astype(F8NP),
            "wv": np.ascontiguousarray(ws["Wv"][:, cols]).astype(BF),
            "wo": wo_bf,
            "bq": np.ascontiguousarray(ws["bq"][cols]),
            "bk": np.ascontiguousarray(ws["bk"][cols]),
            "bv": np.ascontiguousarray(ws["bv"][cols]),
            "bo": ws["bo"],
            "qoff": np.array([[q]], dtype=np.uint32),
        })
    return in_maps


def assemble(results):
    outs = [np.asarray(r["out"], dtype=np.float32) for r in results]
    return np.stack(
        [np.concatenate(outs[b * TP:(b + 1) * TP], axis=0) for b in range(B)]
    )


def _ensure_ntff_hook():
    """bass_utils hard-imports antenv.axon_hooks for trace=True; this image
    lacks it.  Shim it and register the ctypes NTFF hook from trn_boot."""
    import types

    if "antenv.axon_hooks" in sys.modules:
        return
    try:
        import antenv.axon_hooks  # noqa: F401
        return
    except ImportError:
        pass
    mod = types.ModuleType("antenv.axon_hooks")
    mod._hook = None
    mod.set_axon_ntff_profile_hook = lambda h: setattr(mod, "_hook", h)
    mod.get_axon_ntff_profile_hook = lambda: mod._hook
    sys.modules["antenv.axon_hooks"] = mod
    try:
        import antenv
        antenv.axon_hooks = mod
    except Exception:
        pass
    try:
        from trn_agent_boot.trn_boot import _ntff_profile_via_ctypes
        hook = _ntff_profile_via_ctypes("/opt/axon/libaxon_pjrt.so")
        if hook is not None:
            mod._hook = hook
    except Exception:
        pass


_cached_nc = None


def kernel(**inputs):
    global _cached_nc
    _ensure_ntff_hook()
    from concourse.bass_utils import run_bass_kernel_spmd

    if _cached_nc is None:
        _cached_nc = build_bass()
    trace = bool(int(os.environ.get("MHA_TRACE", "0")))
    res = run_bass_kernel_spmd(
        _cached_nc, make_in_maps(inputs), core_ids=list(range(NCORES)),
        trace=trace,
    )
    if trace and res.exec_time_ns is not None:
        print(f"HW exec time: {res.exec_time_ns} ns")
        kernel.last_exec_time_ns = res.exec_time_ns
    return assemble(res.results)



# revision 59
# speedup vs baseline: 1.0132x; 1.0132x over previous
"""Multi-head attention (B=2, N=2048, D=1024, H=16) on 8 trn2 NeuronCores.

Sharding: DP2 (batch) x TP4 (head quarters).  Core c handles batch c//4 and
heads [4*(c%4), 4*(c%4)+4).  Per core:
  - q/k projections as fp8e4 DoubleRow matmuls (weights pre-scaled by 32
    host-side to dodge e4m3 subnormals; x converted bf16->fp8 on the DVE in
    just-in-time blocks).  The v/value path stays bf16 end to end: any fp8
    stage in x->v->z->out passes its ~2% relative error straight through to
    the output, which busts the 2e-2 gate.
  - causal flash-style attention (no max subtraction: scores/32 are tiny so
    exp is safe; softmax denominator comes from a ones-column fused into the
    z-accumulation matmul)
  - a dummy AllGather is issued first so the runtime's global startup
    barrier (tens of us of core-start skew) is absorbed while local DMA and
    compute proceed; then per-(pair, query-block) AllGathers of normalized
    z^T pipeline with attention, each landing as a contiguous [qb] slice of
    one za_all[pr] = [qb, rank, 128, 512] DRAM tensor so a single
    dynamic-offset DMA pulls this core's own query block (no staging pass)
  - output projection for the core's 512-row slice of the sequence
Perf structure: dummy warm-up matmuls ramp the PE p-state through the
input-DMA wait; the PE instruction stream is hand-interleaved (engines
execute in ~emission order): later projection work and the pair-0 output
projection are injected between attention k-tile units, with the pair-0
fillers placed late (0.70+) to ride out peer-skew on the pair-0 gathers.
Host: slices/casts inputs, concatenates the 8 disjoint output slices.
"""

import os
import sys

for _p in ("/opt/trn_rl_repo", "/root/.axon_site/_ro/trn_rl_repo"):
    if os.path.isdir(_p) and _p not in sys.path:
        sys.path.append(_p)

import numpy as np
import ml_dtypes

import concourse.bass as bass
import concourse.mybir as mybir
import concourse.tile as tile
from concourse import bacc

B, N, D, H, HD = 2, 2048, 1024, 16, 64
NCORES, TP = 8, 4
DLOC = D // TP            # 256 local dims (4 heads) per core
P = 128
KT_X = D // P             # 8 contraction tiles for projections
NT = N // P               # 16 n-tiles
QB = 512                  # query block (PSUM bank width in fp32)
NQB = N // QB             # 4
NSLICE = N // TP          # 512 output rows per core
SCALE = 1.0 / 32.0        # 1/sqrt(D)
WS = 32.0                 # fp8 weight pre-scale (avoids e4m3 subnormals)

F32 = mybir.dt.float32
BF16 = mybir.dt.bfloat16
F8 = mybir.dt.float8e4
BF = ml_dtypes.bfloat16
F8NP = ml_dtypes.float8_e4m3
Alu = mybir.AluOpType
Act = mybir.ActivationFunctionType
DR = mybir.MatmulPerfMode.DoubleRow


def build_bass():
    nc = bacc.Bacc("TRN2", num_devices=NCORES)

    xT = nc.dram_tensor("xT", [D, N], BF16, kind="ExternalInput")
    wq = nc.dram_tensor("wq", [D, DLOC], F8, kind="ExternalInput")
    wk = nc.dram_tensor("wk", [D, DLOC], F8, kind="ExternalInput")
    wv = nc.dram_tensor("wv", [D, DLOC], BF16, kind="ExternalInput")
    wo = nc.dram_tensor("wo", [D, D], BF16, kind="ExternalInput")
    bq = nc.dram_tensor("bq", [DLOC], F32, kind="ExternalInput")
    bk = nc.dram_tensor("bk", [DLOC], F32, kind="ExternalInput")
    bv = nc.dram_tensor("bv", [DLOC], F32, kind="ExternalInput")
    bo = nc.dram_tensor("bo", [D], F32, kind="ExternalInput")
    qoff = nc.dram_tensor("qoff", [1, 1], mybir.dt.uint32, kind="ExternalInput")
    # bf16 output: halves the donated zero-buffer upload (start skew) and
    # the tail write DMAs; costs ~0.4% quantization, well inside the gate
    out = nc.dram_tensor("out", [NSLICE, D], BF16, kind="ExternalOutput")

    from concourse.tile_rust import add_dep_helper as _adh
    from concourse.bass import ds

    with tile.TileContext(nc) as tc:
        with (
            tc.tile_pool(name="persist", bufs=1) as persist,
            tc.tile_pool(name="wtp", bufs=4) as wtp,
            tc.tile_pool(name="small", bufs=4) as small,
            tc.tile_pool(name="psA", bufs=2, space="PSUM") as psA,
            tc.tile_pool(name="psS", bufs=2, space="PSUM") as psS,
            tc.tile_pool(name="psZ", bufs=2, space="PSUM") as psZ,
            tc.tile_pool(name="dram", bufs=1, space="DRAM") as dram,
        ):
            # ---- dummy collective first: absorbs the global startup
            # barrier + cc-ring init (up to ~100us of core-start skew)
            # while this core's local DMA + compute proceed, so the real
            # gathers later don't inherit that latency ----
            dum_in = dram.tile([1, 8], BF16, name="dum_in")
            dum_out = dram.tile([TP, 8], BF16, name="dum_out")
            nc.gpsimd.collective_compute(
                "AllGather",
                Alu.bypass,
                replica_groups=[[0, 1, 2, 3], [4, 5, 6, 7]],
                ins=[dum_in[:].opt()],
                outs=[dum_out[:].opt()],
            )

            # ---- PE warm-up: dummy matmuls with no input dependencies
            # ramp the PE p-state while the input DMAs stream; keep it
            # short so the HAM activity budget is spent on real work ----
            warm_l = persist.tile([P, P], BF16)
            nc.vector.memset(warm_l, 0.0)
            warm_r = persist.tile([P, QB], BF16)
            nc.vector.memset(warm_r, 0.0)
            for _ in range(24):
                psw = psA.tile([P, QB], F32, tag="proj", name="psw")
                nc.tensor.matmul(psw, lhsT=warm_l, rhs=warm_r,
                                 start=True, stop=True)

            # preload the EXP activation table before attention needs it
            exp_warm = small.tile([1, 8], F32)
            nc.vector.memset(exp_warm, 0.0)
            exp_wout = small.tile([1, 8], F32)
            nc.scalar.activation(exp_wout, exp_warm, Act.Exp)

            # ---- input loads: wq/wk + the first x block lead the sync
            # queue (they gate the first projection); biases follow ----
            wq_sb = persist.tile([P, KT_X, DLOC], F8)
            wk_sb = persist.tile([P, KT_X, DLOC], F8)
            wq_r = wq[:].rearrange("(kt p) m -> p kt m", p=P)
            wk_r = wk[:].rearrange("(kt p) m -> p kt m", p=P)
            for h in range(2):
                kt4 = slice(h * 4, h * 4 + 4)
                nc.sync.dma_start(wq_sb[:, kt4], wq_r[:, kt4])
                nc.sync.dma_start(wk_sb[:, kt4], wk_r[:, kt4])

            xT_sb = persist.tile([P, KT_X, N], BF16)
            xT_f8 = persist.tile([P, KT_X, N], F8)
            xT_r = xT[:].rearrange("(kt p) m -> p kt m", p=P)

            def load_x_block(qc, chunks=1):
                w = QB // chunks
                for ch in range(chunks):
                    lo = qc * QB + ch * w
                    nc.sync.dma_start(
                        xT_sb[:, :, lo:lo + w], xT_r[:, :, lo:lo + w])

            def conv_x_block(qc):
                # fp8 shadow of x for the DoubleRow q/k projections (the
                # v/value path reads the bf16 original for precision).
                # On DVE, emitted just-in-time before the first consumer
                # so the queue never stalls on a pending x DMA.
                nc.vector.tensor_copy(
                    xT_f8[:, :, qc * QB:(qc + 1) * QB],
                    xT_sb[:, :, qc * QB:(qc + 1) * QB])

            load_x_block(0, chunks=4)
            conv_x_block(0)

            bq_sb = small.tile([P, 2], F32)
            nc.sync.dma_start(bq_sb, bq[:].rearrange("(t p) -> p t", p=P))
            bqs_sb = persist.tile([P, 2], F32)
            nc.vector.tensor_scalar_mul(bqs_sb, bq_sb, SCALE)
            bk_sb = persist.tile([P, 2], F32)
            nc.sync.dma_start(bk_sb, bk[:].rearrange("(t p) -> p t", p=P))
            bv_row = small.tile([1, DLOC], F32)
            nc.sync.dma_start(bv_row, bv[:].rearrange("(a d) -> a d", a=1))
            bv_bc = persist.tile([P, DLOC], F32)
            nc.gpsimd.partition_broadcast(bv_bc, bv_row)
            bo_row = small.tile([1, D], F32)
            nc.sync.dma_start(bo_row, bo[:].rearrange("(a d) -> a d", a=1))
            bo_bc = persist.tile([P, D], F32)
            nc.gpsimd.partition_broadcast(bo_bc, bo_row)

            wv_sb = persist.tile([P, KT_X, DLOC], BF16)
            nc.sync.dma_start(wv_sb, wv[:].rearrange("(kt p) m -> p kt m", p=P))
            for qc in range(1, NQB):
                load_x_block(qc)
            wo_sb = persist.tile([P, KT_X, D], BF16)
            nc.sync.dma_start(wo_sb, wo[:].rearrange("(kt p) m -> p kt m", p=P))

            # 128x128 lower-triangle mask: tri[kk, j] = 1 iff j >= kk
            mask_sb = persist.tile([P, P], BF16)
            nc.gpsimd.memset(mask_sb, 1.0)
            nc.gpsimd.affine_select(
                out=mask_sb,
                in_=mask_sb,
                compare_op=Alu.is_ge,
                fill=0.0,
                base=0,
                pattern=[[1, P]],
                channel_multiplier=-1,
            )

            qT_sb = persist.tile([P, 2, N], BF16)
            kT_sb = persist.tile([P, 2, N], BF16)
            v_sb = persist.tile([P, NT, 4 * 65], BF16)
            # ones columns for the denominators are preset inside fill0
            # (keeps the DVE queue free for the x fp8 convert at start)

            # ---- QKV projection emit-chunks (filler units) ----
            def qk_unit(pr, qc, which):
                if which == "q":
                    psq = psA.tile([P, QB], F32, tag="proj", name="psq")
                    for k2 in range(KT_X // 2):
                        nc.tensor.matmul(
                            psq,
                            lhsT=wq_sb[:, 2 * k2:2 * k2 + 2,
                                       pr * P:(pr + 1) * P],
                            rhs=xT_f8[:, 2 * k2:2 * k2 + 2,
                                      qc * QB:(qc + 1) * QB],
                            start=(k2 == 0),
                            stop=(k2 == KT_X // 2 - 1),
                            perf_mode=DR,
                        )
                    nc.vector.tensor_scalar(
                        qT_sb[:, pr, qc * QB:(qc + 1) * QB],
                        psq, SCALE / WS, bqs_sb[:, pr:pr + 1],
                        Alu.mult, Alu.add,
                    )
                else:
                    psk = psA.tile([P, QB], F32, tag="proj", name="psk")
                    for k2 in range(KT_X // 2):
                        nc.tensor.matmul(
                            psk,
                            lhsT=wk_sb[:, 2 * k2:2 * k2 + 2,
                                       pr * P:(pr + 1) * P],
                            rhs=xT_f8[:, 2 * k2:2 * k2 + 2,
                                      qc * QB:(qc + 1) * QB],
                            start=(k2 == 0),
                            stop=(k2 == KT_X // 2 - 1),
                            perf_mode=DR,
                        )
                    nc.vector.tensor_scalar(
                        kT_sb[:, pr, qc * QB:(qc + 1) * QB],
                        psk, 1.0 / WS, bk_sb[:, pr:pr + 1],
                        Alu.mult, Alu.add,
                    )

            def v_unit(nt):
                psv_full = psA.tile([P, QB], F32, tag="proj", name="psv")
                psv = psv_full[:, :DLOC]
                for kt in range(KT_X):
                    nc.tensor.matmul(
                        psv,
                        lhsT=xT_sb[:, kt, nt * P:(nt + 1) * P],
                        rhs=wv_sb[:, kt, :],
                        start=(kt == 0),
                        stop=(kt == KT_X - 1),
                    )
                nc.vector.tensor_tensor(
                    v_sb[:, nt].rearrange("p (h x) -> p h x", x=65)[:, :, 0:64],
                    psv.rearrange("p (h x) -> p h x", x=64),
                    bv_bc.rearrange("p (h x) -> p h x", x=64),
                    Alu.add,
                )

            # per-core query-block index for the output projection
            qoff_sb = small.tile([1, 1], mybir.dt.uint32)
            nc.sync.dma_start(qoff_sb, qoff[:])
            qregs = nc.alloc_registers()
            nc.regs_load(qregs, qoff_sb[0:1, 0:1])
            qoff_sv = nc.snap(qregs, donate=True)

            # collective buffers.  zin[pr]: [qb, 128, 512] per-block slabs.
            # Per-(pr,qb) AllGathers pipeline with attention; each output
            # lands as a contiguous [qb] slice of za_all[pr]:
            # [qb, rank, 128, 512], so a single dynamic-offset DMA can
            # later pull this core's own query block (no staging pass).
            zin = [dram.tile([NQB, P, QB], BF16, name=f"zin{pr}")
                   for pr in range(2)]
            za_all = [dram.tile([NQB, TP, P, QB], BF16, name=f"za_all{pr}")
                      for pr in range(2)]

            RG = [[0, 1, 2, 3], [4, 5, 6, 7]]

            def gather(pr, qb):
                return nc.gpsimd.collective_compute(
                    "AllGather",
                    Alu.bypass,
                    replica_groups=RG,
                    ins=[zin[pr][qb].opt()],
                    outs=[za_all[pr][qb].opt()],
                )

            # ---- attention for one head pair, with filler injection.
            # `sched` is a sorted list of (when, fn): fn is emitted once the
            # fraction of emitted k-tile slots reaches `when`.  `on_norm` is
            # called (to emit collective triggers / staging) right after a
            # query block's zin writes. ----
            def attention(pr, sched, on_norm):
                nf = len(sched)
                total_slots = sum((qb + 1) * 4 for qb in range(NQB))
                fi = 0
                slot_i = 0
                for qb in range(NQB):
                    kt_max = (qb + 1) * 4
                    zps = [psZ.tile([65, QB], F32, tag="z", name=f"zp{hi}")
                           for hi in range(2)]
                    pending_z = []
                    sp_tiles = wt_tiles = g_os = None
                    for kt in range(kt_max):
                        g, slot = divmod(kt, 2)
                        diag = kt >= qb * 4
                        o = kt * P - qb * QB if diag else 0
                        if slot == 0:
                            sp_tiles = [
                                psS.tile([P, 2, QB], F32, tag="score",
                                         name=f"sp{hi}")
                                for hi in range(2)
                            ]
                            wt_tiles = [
                                wtp.tile([P, 2, QB], BF16, tag="wt",
                                         name=f"wt{hi}")
                                for hi in range(2)
                            ]
                            g_os = []
                        g_os.append(o)
                        s_insts = []
                        for hi in range(2):
                            si = nc.tensor.matmul(
                                sp_tiles[hi][:, slot, o:QB],
                                lhsT=kT_sb[hi * 64:(hi + 1) * 64, pr,
                                           kt * P:(kt + 1) * P],
                                rhs=qT_sb[hi * 64:(hi + 1) * 64, pr,
                                          qb * QB + o:(qb + 1) * QB],
                                start=True,
                                stop=True,
                                tile_position=(hi * 64, 0),
                            )
                            s_insts.append(si)
                        for args in pending_z:
                            zi = nc.tensor.matmul(**args)
                            _adh(zi.ins, s_insts[-1].ins, sync=False,
                                 reason="z after score pair")
                        pending_z = []
                        # inject scheduled filler units between k-tile slots
                        slot_i += 1
                        frac = slot_i / total_slots
                        while fi < nf and sched[fi][0] <= frac:
                            sched[fi][1]()
                            fi += 1

                        if slot == 1:
                            # one exp per pair even when the two slots
                            # have different diagonal offsets: the strip
                            # [min(o), max(o)) of the later slot holds
                            # exp(stale scores) that no z matmul or mask
                            # ever reads, and one instruction saves the
                            # per-op PSUM-access/decode overhead at the
                            # joints where z waits on exp
                            o1 = min(g_os)
                            for hi in range(2):
                                nc.scalar.activation(
                                    wt_tiles[hi][:, :, o1:QB],
                                    sp_tiles[hi][:, :, o1:QB], Act.Exp)
                                for s, oo in enumerate(g_os):
                                    if g * 2 + s >= qb * 4:
                                        nc.vector.tensor_tensor(
                                            wt_tiles[hi][:, s, oo:oo + P],
                                            wt_tiles[hi][:, s, oo:oo + P],
                                            mask_sb,
                                            Alu.mult,
                                        )
                            for s, oo in enumerate(g_os):
                                k_abs = g * 2 + s
                                for hi in range(2):
                                    pending_z.append(dict(
                                        out=zps[hi][:, oo:QB],
                                        lhsT=v_sb[:, k_abs,
                                                  (2 * pr + hi) * 65:
                                                  (2 * pr + hi + 1) * 65],
                                        rhs=wt_tiles[hi][:, s, oo:QB],
                                        start=(k_abs == 0),
                                        stop=(k_abs == kt_max - 1),
                                        skip_group_check=True,
                                    ))
                    for args in pending_z:
                        nc.tensor.matmul(**args)
                    for hi in range(2):
                        # denominator -> SBUF (approx_fast misbehaves on a
                        # PSUM source), reciprocal, gpsimd partition
                        # broadcast, normalize, zin write on the sync queue.
                        den = small.tile([1, QB], F32, tag="den", name="den")
                        nc.vector.tensor_copy(den, zps[hi][64:65, :])
                        recip = small.tile([1, QB], F32, tag="recip",
                                           name="recip")
                        nc.vector.reciprocal_approx_fast(recip, den)
                        rb = small.tile([64, QB], F32, tag="rb", name="rb")
                        nc.gpsimd.partition_broadcast(rb, recip)
                        zn = small.tile([64, QB], BF16, tag="zn", name="zn")
                        nc.vector.tensor_tensor(zn, zps[hi][0:64, :], rb,
                                                Alu.mult)
                        nc.sync.dma_start(
                            zin[pr][qb, hi * 64:(hi + 1) * 64, :], zn)
                    on_norm(pr, qb)
                while fi < nf:
                    sched[fi][1]()
                    fi += 1

            # ---- output projection halves ----
            zg_sb = [persist.tile([P, TP, QB], BF16, name=f"zg{pr}")
                     for pr in range(2)]
            stage_sb = persist.tile([P, NSLICE // P, D // QB, QB], F32)

            def select_zg(pr):
                # dynamic query-block select straight out of the gather
                # outputs (sync queue; gated on the collectives by the
                # tile dependency tracker)
                nc.sync.dma_start(
                    zg_sb[pr],
                    za_all[pr].rearrange("qb r p n -> p qb r n")[
                        :, ds(qoff_sv, 1), :, :].opt(keep_dims=[0, 2, 3]),
                )

            def oproj_unit(pr, mt, oc):
                pso = psA.tile([P, QB], F32, tag="proj", name="pso")
                for r in range(TP):
                    nc.tensor.matmul(
                        pso,
                        lhsT=zg_sb[pr][:, r, mt * P:(mt + 1) * P],
                        rhs=wo_sb[:, pr * TP + r, oc * QB:(oc + 1) * QB],
                        start=(r == 0),
                        stop=(r == TP - 1),
                    )
                if pr == 0:
                    nc.vector.tensor_tensor(
                        stage_sb[:, mt, oc], pso,
                        bo_bc[:, oc * QB:(oc + 1) * QB], Alu.add)
                else:
                    osb = small.tile([P, QB], BF16, tag="osb", name="osb")
                    nc.vector.tensor_tensor(
                        osb, pso, stage_sb[:, mt, oc], Alu.add)
                    nc.sync.dma_start(
                        out[mt * P:(mt + 1) * P, oc * QB:(oc + 1) * QB], osb)

            # ---- emission schedule ----
            qk_unit(0, 0, "q")
            qk_unit(0, 0, "k")

            # fill0 placement: v tile j must land before z consumes it
            # (qb = j//4); q/k for block j before attention reaches qb=j;
            # pair-1 q/k late.
            fill0 = (
                [(0.0005, lambda: nc.vector.memset(v_sb, 1.0))]
                + [(0.001 + 0.018 * nt, lambda nt=nt: v_unit(nt))
                   for nt in range(4)]
                + [
                    (0.06, lambda: conv_x_block(1)),
                    (0.08, lambda: qk_unit(0, 1, "q")),
                    (0.10, lambda: qk_unit(0, 1, "k")),
                    (0.12, lambda: v_unit(4)),
                    (0.14, lambda: v_unit(5)),
                    (0.16, lambda: v_unit(6)),
                    (0.18, lambda: v_unit(7)),
                    (0.19, lambda: conv_x_block(2)),
                    (0.21, lambda: qk_unit(0, 2, "q")),
                    (0.24, lambda: qk_unit(0, 2, "k")),
                    (0.28, lambda: v_unit(8)),
                    (0.31, lambda: v_unit(9)),
                    (0.34, lambda: v_unit(10)),
                    (0.37, lambda: v_unit(11)),
                    (0.40, lambda: conv_x_block(3)),
                    (0.42, lambda: qk_unit(0, 3, "q")),
                    (0.46, lambda: qk_unit(0, 3, "k")),
                    (0.50, lambda: v_unit(12)),
                    (0.53, lambda: v_unit(13)),
                    (0.56, lambda: v_unit(14)),
                    (0.59, lambda: v_unit(15)),
                    (0.66, lambda: qk_unit(1, 0, "q")),
                    (0.72, lambda: qk_unit(1, 0, "k")),
                    (0.80, lambda: qk_unit(1, 1, "q")),
                    (0.88, lambda: qk_unit(1, 1, "k")),
                ]
            )

            def on_norm0(pr, qb):
                gather(0, qb)

            attention(0, fill0, on_norm0)

            fill1 = [
                (0.00, lambda: qk_unit(1, 2, "q")),
                (0.05, lambda: qk_unit(1, 2, "k")),
                (0.22, lambda: qk_unit(1, 3, "q")),
                (0.28, lambda: qk_unit(1, 3, "k")),
                # pr0's last gather rides on peer skew; give it ~2/3 of
                # pr1's attention before the fillers need its output
                (0.66, lambda: select_zg(0)),
            ] + [
                # late placement: by 0.80 of pr1's attention the pr0
                # gathers have cleared even a slow peer, so these never
                # block the in-order PE queue mid-attention
                (0.80 + 0.025 * i,
                 lambda mt=mt, oc=oc: oproj_unit(0, mt, oc))
                for i, (mt, oc) in enumerate(
                    (mt, oc) for mt in range(NSLICE // P)
                    for oc in range(D // QB))
            ]

            def on_norm1(pr, qb):
                gather(1, qb)

            attention(1, fill1, on_norm1)
            select_zg(1)
            # keep the PE p-state hot through the tail gather wait so the
            # final output projection runs at full clock, not half
            for _ in range(45):
                psw = psA.tile([P, QB], F32, tag="proj", name="psw")
                nc.tensor.matmul(psw, lhsT=warm_l, rhs=warm_r,
                                 start=True, stop=True)
            for mt in range(NSLICE // P):
                for oc in range(D // QB):
                    oproj_unit(1, mt, oc)
    nc.compile()
    return nc


def make_in_maps(inputs):
    x = np.asarray(inputs["inputs"], dtype=np.float32)
    ws = {k: np.asarray(inputs[k], dtype=np.float32) for k in
          ("Wq", "Wk", "Wv", "Wo", "bq", "bk", "bv", "bo")}
    # permute Wo rows to the kernel's k-tile order: kt = pr*4 + rank maps to
    # original rows [rank*256 + pr*128, +128)
    wo_perm = (ws["Wo"].reshape(TP, 2, P, D).transpose(1, 0, 2, 3)
               .reshape(D, D))
    wo_bf = np.ascontiguousarray(wo_perm).astype(BF)
    xT_bf = [np.ascontiguousarray(x[b].T).astype(BF) for b in range(B)]
    in_maps = []
    for c in range(NCORES):
        b, q = c // TP, c % TP
        cols = slice(q * DLOC, (q + 1) * DLOC)
        in_maps.append({
            "xT": xT_bf[b],
            "wq": np.ascontiguousarray(ws["Wq"][:, cols] * WS).astype(F8NP),
            "wk": np.ascontiguousarray(ws["Wk"][:, cols] * WS).astype(F8NP),
            "wv": np.ascontiguousarray(ws["Wv"][:, cols]).astype(BF),
            "wo": wo_bf,
            "bq": np.ascontiguousarray(ws["bq"][cols]),
            "bk": np.ascontiguousarray(ws["bk"][cols]),
            "bv": np.ascontiguousarray(ws["bv"][cols]),
            "bo": ws["bo"],
            "qoff": np.array([[q]], dtype=np.uint32),
        })
    return in_maps


def assemble(results):
    outs = [np.asarray(r["out"], dtype=np.float32) for r in results]
    return np.stack(
        [np.concatenate(outs[b * TP:(b + 1) * TP], axis=0) for b in range(B)]
    )


def _ensure_ntff_hook():
    """bass_utils hard-imports antenv.axon_hooks for trace=True; this image
    lacks it.  Shim it and register the ctypes NTFF hook from trn_boot."""
    import types

    if "antenv.axon_hooks" in sys.modules:
        return
    try:
        import antenv.axon_hooks  # noqa: F401
        return
    except ImportError:
        pass
    mod = types.ModuleType("antenv.axon_hooks")
    mod._hook = None
    mod.set_axon_ntff_profile_hook = lambda h: setattr(mod, "_hook", h)
    mod.get_axon_ntff_profile_hook = lambda: mod._hook
    sys.modules["antenv.axon_hooks"] = mod
    try:
        import antenv
        antenv.axon_hooks = mod
    except Exception:
        pass
    try:
        from trn_agent_boot.trn_boot import _ntff_profile_via_ctypes
        hook = _ntff_profile_via_ctypes("/opt/axon/libaxon_pjrt.so")
        if hook is not None:
            mod._hook = hook
    except Exception:
        pass


_cached_nc = None


def kernel(**inputs):
    global _cached_nc
    _ensure_ntff_hook()
    from concourse.bass_utils import run_bass_kernel_spmd

    if _cached_nc is None:
        _cached_nc = build_bass()
    trace = bool(int(os.environ.get("MHA_TRACE", "0")))
    res = run_bass_kernel_spmd(
        _cached_nc, make_in_maps(inputs), core_ids=list(range(NCORES)),
        trace=trace,
    )
    if trace and res.exec_time_ns is not None:
        print(f"HW exec time: {res.exec_time_ns} ns")
        kernel.last_exec_time_ns = res.exec_time_ns
    return assemble(res.results)

